# revision 33
# baseline (speedup 1.0000x reference)
"""Bilinear pooling kernel for 8 Trainium2 NeuronCores (Bass/Tile).

Math (matches the jax reference):
  x = concat([x1, x2, x3], channel) -> (B=64, M=147, L=3136)
  phi_b = x_b @ x_b.T                              (147, 147), symmetric
  phi = sign(phi) * sqrt(|phi| + EPS)              (signed sqrt)
  phi = phi / sqrt(sum(phi^2 + EPS) + 1.0)         (per-batch normalize)
  h = phi_vec @ fc0_w.T + fc0_b                    (64, 1024)
  y = h @ fc1_w.T + fc1_b                          (64, 64)
  logits = y @ fc2_w.T + fc2_b                     (64, 4)
  merged = softmax(concat([logits, x11, x21, x31]))
  x_merge = merged @ cls_w.T + cls_b               (64, 4)
  returns (logits, x_merge)

Key structural tricks:
  * fc0, fc1, fc2 are linear with no nonlinearity between them (dropout
    is identity at inference), so all three are fused HOST-SIDE:
    Wf = fc2_w @ fc1_w @ fc0_w (4 x 21609, ~151 KB fp16) is replicated on
    every core, and each core computes logits / softmax / x_merge for its
    OWN 8 batches entirely locally.  There are NO collectives at all --
    the host concatenates the per-core (8, 4) outputs.  This matters
    because the CC stream is blocked by a ~58us comm-init barrier and
    each collective op costs 11-30us regardless of payload size.
  * phi is symmetric: only the top 128x147 block (A) and the 19x19 corner
    are computed; Wf is folded host-side (WfA[m,n] += Wf[n*147+m] for
    n>=128) so no transpose of phi is ever needed and the bilinear pass
    does 147+19 instead of 2*147 matmul columns per l-chunk.
  * all big DMAs are partition-contiguous (host pre-transposes): each
    transfer is 128 descriptors of >=512B instead of thousands of ~280B.
  * the softmax division is deferred past the (linear) cls matmul: the
    exp row-sum rides as a 17th feature into wct's bias row and x_merge
    is rescaled at the very end, shortening the serial tail chain.

Distribution:
  phase 0: preload fused weights + tail constants (overlaps phase 1)
  phase 1: batch-parallel bilinear+signed-sqrt+normalize (8 batches/core),
           normalized phi written straight into SBUF (no DRAM roundtrip)
  phase 2: fused fc0+fc1+fc2: 147+19+1 accumulating chunk matmuls ->
           logits (8, 4)
  phase 3: local tail (softmax, cls) for the core's 8 batches; each core
           writes its own 8 output rows, the host concatenates
"""

import sys

sys.path.insert(0, "/opt/trn_rl_repo")

import numpy as np

import concourse.bass as bass
import concourse.tile as tile
from concourse import masks, mybir
from concourse.bass_utils import run_bass_kernel_spmd
import bass_rust
from bass_rust import ScopedClock

# ---------------------------------------------------------------------------
# Workaround: this toolchain's walrus accepts only ONE semaphore wait per
# instruction, but Tile can attach several.  Split excess waits onto
# same-engine nops placed immediately before the instruction (same engine
# => executed in order, so synchronization semantics are unchanged).
# ---------------------------------------------------------------------------
_MAX_WAITS = 1
_ws_counter = [0]


def _split_excess_waits(obb):
    for bb, insts in list(obb.items()):
        new_list = []
        for inst in insts:
            info = inst.sync_info
            if info is not None and len(info.on_wait) > _MAX_WAITS:
                waits = list(info.on_wait)
                excess = waits[:-_MAX_WAITS]
                keep = waits[-_MAX_WAITS:]
                for i in range(0, len(excess), _MAX_WAITS):
                    _ws_counter[0] += 1
                    nop = mybir.InstNoOp(
                        name=f"WS-{_ws_counter[0]}",
                        sync_info=bass_rust.SyncInfo(
                            on_wait=excess[i : i + _MAX_WAITS], on_update=[]
                        ),
                        bass_nofuse=True,
                        engine=inst.engine,
                    )
                    new_list.append(nop)
                inst.sync_info = bass_rust.SyncInfo(
                    on_wait=keep, on_update=list(info.on_update)
                )
            new_list.append(inst)
        obb[bb] = new_list


_RealTCW = tile.TileClockWait


class _TCWWrapper:
    def __init__(self, *args, **kwargs):
        self._inner = _RealTCW(*args, **kwargs)
        self._obb = (
            args[1] if len(args) > 1 else kwargs["ordered_instructions_by_block"]
        )

    def __getattr__(self, name):
        return getattr(self._inner, name)

    def assign_waits(self, bb_name):
        self._inner.assign_waits(bb_name)
        _split_excess_waits(self._obb)


tile.TileClockWait = _TCWWrapper


def _split_drain_and_barrier(self, tick_clock, wait_clock):
    nc = self.nc
    drain_inst = nc.sync.drain()
    wait_clock.add_sem_waits(
        drain_inst.ins, ScopedClock({None: tick_clock.global_clock})
    )
    info = drain_inst.ins.sync_info
    if info is not None and len(info.on_wait) > _MAX_WAITS:
        waits = list(info.on_wait)
        drain_inst.ins.sync_info = bass_rust.SyncInfo(
            on_wait=waits[:_MAX_WAITS], on_update=list(info.on_update)
        )
        rest = waits[_MAX_WAITS:]
        while rest:
            chunk, rest = rest[:_MAX_WAITS], rest[_MAX_WAITS:]
            nop_inst = nc.sync.nop(nofuse=True, hint="tail_drain_split")
            nop_inst.ins.sync_info = bass_rust.SyncInfo(on_wait=chunk, on_update=[])
    nc.all_engine_barrier()
    assert self.sems is not None
    popped = nc._tile_sem_poison_stack.pop()
    assert popped is self._sem_poison
    nc.clear_and_free_semaphores(list(self.sems.allocated().values()))
    nc.all_engine_barrier()


tile.TileContext._drain_and_barrier = _split_drain_and_barrier

# ---------------------------------------------------------------------------
# Problem constants (hardcoded per the spec)
# ---------------------------------------------------------------------------
N_CORES = 8
CORE_IDS = list(range(N_CORES))
B = 64
B_LOC = B // N_CORES  # 8 batches per core
C = 49
L = 3136  # 56*56
LC = 25  # l-chunks of 128 (3200 rows, last 64 zero-padded)
M = 147  # 3*49 channels
MA = 128  # top block rows
MB = M - MA  # 19 corner rows
MM = M * M  # 21609
O0 = 1024  # fc0 out features
HID = 64  # fc1 out features
CLS = 4
EPS = 1e-8
# normalizer constant: sum(phi_ss^2 + EPS) + 1.0 == sum|phi| + 2*MM*EPS + 1.0
NORM_C = float(2 * MM * EPS + 1.0)

F32 = mybir.dt.float32
MM_DT = mybir.dt.float16


def _build_nc():
    nc = bass.Bass()

    # -- external I/O ------------------------------------------------------
    # x arrives host-side concatenated, zero-padded to 3200 l-rows, and
    # pre-transposed to [b][p=128][lc=25][m=147] so each per-batch DMA is
    # 128 descriptors x 7350 B.
    xall_d = nc.dram_tensor("xall", [B_LOC, 128, LC, M], MM_DT, kind="ExternalInput")
    # per-core slices of x11/x21/x31 (this core's 8 batches)
    x11_d = nc.dram_tensor("x11", [B_LOC, CLS], F32, kind="ExternalInput")
    x21_d = nc.dram_tensor("x21", [B_LOC, CLS], F32, kind="ExternalInput")
    x31_d = nc.dram_tensor("x31", [B_LOC, CLS], F32, kind="ExternalInput")
    # fused fc2@fc1@fc0 weights (all linear, dropout = identity), folded:
    #   wA[m, n, y] = Wf[y, m*147+n] (+ Wf[y, n*147+m] for n >= 128)
    #   w4[a, b4, y] = Wf[y, (128+a)*147 + 128+b4]
    wA_d = nc.dram_tensor("wA", [MA, M, CLS], MM_DT, kind="ExternalInput")
    w4_d = nc.dram_tensor("w4", [MB, MB, CLS], MM_DT, kind="ExternalInput")
    bf_d = nc.dram_tensor("bf", [1, CLS], F32, kind="ExternalInput")
    wct_d = nc.dram_tensor("wct", [4 * CLS + 1, CLS], F32, kind="ExternalInput")
    # each core writes only its own 8 batches; the host concatenates
    logits_d = nc.dram_tensor("logits", [B_LOC, CLS], F32, kind="ExternalOutput")
    xmerge_d = nc.dram_tensor("x_merge", [B_LOC, CLS], F32, kind="ExternalOutput")

    with tile.TileContext(nc) as tc:
        with tc.tile_pool(name="const", bufs=1) as const:
            # -- constants ----------------------------------------------
            identf = const.tile([128, 128], F32)
            masks.make_identity(nc, identf[:])
            ones_col = const.tile([128, 128], F32)
            nc.gpsimd.memset(ones_col[:], 1.0)
            eps_col = const.tile([128, 1], F32)
            nc.gpsimd.memset(eps_col[:], EPS)
            normc_col = const.tile([128, 1], F32)
            nc.gpsimd.memset(normc_col[:], NORM_C)

            # normalized phi blocks, SBUF-resident across phases 1-2
            pallA = const.tile([MA, B_LOC, M], MM_DT)  # [p=m, bl, n]
            pallB = const.tile([MB, B_LOC, MB], MM_DT)  # [p=a, bl, b4]

            # ===========================================================
            # phase 1 + 0: per-batch bilinear pipeline; weight preloads
            # are issued after the first xt DMAs so x gets DMA priority
            # ===========================================================
            with tc.tile_pool(name="xt", bufs=5) as xt_pool, tc.tile_pool(
                name="p1sb", bufs=2
            ) as sb, tc.tile_pool(
                name="p1ps", bufs=2, space="PSUM"
            ) as ps, nc.named_scope("p1_bilinear"):

                LH = 13  # batch-0 first-half l-chunks

                def p1_mains(b):
                    if b == 0:
                        # batch 0 is on the startup critical path: load it in
                        # two halves so the A-pass starts after the first
                        xta = xt_pool.tile([128, LH, M], MM_DT, tag="xta")
                        xtb = xt_pool.tile([128, LC - LH, M], MM_DT, tag="xtb")
                        nc.sync.dma_start(xta[:], xall_d[0][:, 0:LH, :])
                        nc.sync.dma_start(xtb[:], xall_d[0][:, LH:LC, :])

                        def sl(lc):
                            return (
                                xta[:, lc, :] if lc < LH else xtb[:, lc - LH, :]
                            )
                    else:
                        xt = xt_pool.tile([128, LC, M], MM_DT, tag="xt")
                        nc.sync.dma_start(xt[:], xall_d[b])

                        def sl(lc):
                            return xt[:, lc, :]

                    # A block: phi[0:128, 0:147]; corner: phi[128:147, 128:147]
                    pA = ps.tile([MA, M], F32, tag="pA", bufs=3)
                    pB2 = ps.tile([MB, MB], F32, tag="pB2", bufs=3)
                    for lc in range(LC):
                        nc.tensor.matmul(
                            pA[:],
                            sl(lc)[:, 0:MA],
                            sl(lc),
                            start=(lc == 0),
                            stop=(lc == LC - 1),
                        )
                    for lc in range(LC):
                        nc.tensor.matmul(
                            pB2[:],
                            sl(lc)[:, MA:M],
                            sl(lc)[:, MA:M],
                            start=(lc == 0),
                            stop=(lc == LC - 1),
                        )
                    return pA, pB2

                def p1_norm(b, pA, pB2):
                    # signed sqrt pieces + |phi| row sums (accumulated on ACT)
                    sgnA = sb.tile([MA, M], F32, tag="sgnA")
                    absA = sb.tile([MA, M], F32, tag="absA")
                    rsA = sb.tile([MA, 1], F32, tag="rsA")
                    nc.scalar.activation(
                        sgnA[:], pA[:], mybir.ActivationFunctionType.Sign
                    )
                    nc.scalar.activation(
                        absA[:],
                        pA[:],
                        mybir.ActivationFunctionType.Abs,
                        accum_out=rsA[:],
                    )
                    sgnB2 = sb.tile([MB, MB], F32, tag="sgnB2")
                    absB2 = sb.tile([MB, MB], F32, tag="absB2")
                    rsB2 = sb.tile([MB, 1], F32, tag="rsB2")
                    nc.scalar.activation(
                        sgnB2[:], pB2[:], mybir.ActivationFunctionType.Sign
                    )
                    nc.scalar.activation(
                        absB2[:],
                        pB2[:],
                        mybir.ActivationFunctionType.Abs,
                        accum_out=rsB2[:],
                    )
                    # S2 columns counted twice (symmetric image lives in S3)
                    rsA2 = sb.tile([MA, 1], F32, tag="rsA2")
                    nc.vector.reduce_sum(
                        rsA2[:], absA[:, MA:M], axis=mybir.AxisListType.X
                    )
                    rsT = sb.tile([MA, 1], F32, tag="rsT")
                    nc.vector.tensor_add(rsT[:], rsA[:], rsA2[:])
                    nc.vector.tensor_add(
                        rsT[0:MB], rsT[0:MB], rsB2[:]
                    )

                    # cross-partition sum + broadcast in one accumulation
                    # group: bc[m] = sum_k ones[k, m] * rs[k]
                    bc = ps.tile([128, 1], F32, tag="bc")
                    nc.tensor.matmul(
                        bc[:], ones_col[:, :], rsT[:], start=True, stop=True
                    )

                    # ss = sign * sqrt(|phi| + EPS)
                    sqA = sb.tile([MA, M], F32, tag="sqA")
                    nc.scalar.activation(
                        sqA[:],
                        absA[:],
                        mybir.ActivationFunctionType.Sqrt,
                        bias=eps_col[:],
                    )
                    ssA = sb.tile([MA, M], F32, tag="ssA")
                    nc.vector.tensor_mul(ssA[:], sqA[:], sgnA[:])
                    sqB2 = sb.tile([MB, MB], F32, tag="sqB2")
                    nc.scalar.activation(
                        sqB2[:],
                        absB2[:],
                        mybir.ActivationFunctionType.Sqrt,
                        bias=eps_col[0:MB],
                    )
                    ssB2 = sb.tile([MB, MB], F32, tag="ssB2")
                    nc.vector.tensor_mul(ssB2[:], sqB2[:], sgnB2[:])

                    # scale = 1 / sqrt(total + NORM_C)
                    inv = sb.tile([128, 1], F32, tag="inv")
                    nc.scalar.activation(
                        inv[:],
                        bc[:],
                        mybir.ActivationFunctionType.Sqrt,
                        bias=normc_col[:],
                    )
                    scl = sb.tile([128, 1], F32, tag="scl")
                    nc.vector.reciprocal(scl[:], inv[:])

                    # normalized phi straight into the SBUF-resident blocks
                    nc.vector.tensor_scalar_mul(
                        pallA[:, b, :], ssA[:], scl[0:MA]
                    )
                    nc.vector.tensor_scalar_mul(
                        pallB[:, b, :], ssB2[:], scl[0:MB]
                    )

                # 2-batch software pipeline: batch b's norm chain is issued
                # after batch b+2's matmuls so the PE stream never stalls
                # waiting on the ACT/DVE normalizer chain (pA bufs=3 keeps
                # three batches' PSUM alive)
                pend = []
                for b in range(B_LOC):
                    pend.append((b, *p1_mains(b)))
                    if len(pend) > 2:
                        p1_norm(*pend.pop(0))
                    if b == 3:
                        # weight preloads issued behind the first xt DMAs
                        # (tiny now: the fused Wf is only ~151 KB)
                        w_sb = const.tile([MA, M, CLS], MM_DT)
                        nc.scalar.dma_start(w_sb[:], wA_d[:])
                        w4_sb = const.tile([MB, MB, CLS], MM_DT)
                        nc.scalar.dma_start(w4_sb[:], w4_d[:])
                        bf_sb = const.tile([1, CLS], F32)
                        nc.scalar.dma_start(bf_sb[:], bf_d[:])
                        wc_sb = const.tile([4 * CLS + 1, CLS], F32)
                        nc.scalar.dma_start(wc_sb[:], wct_d[:])
                        xm1_sb = const.tile([B_LOC, CLS], F32)
                        nc.scalar.dma_start(xm1_sb[:], x11_d[:])
                        xm2_sb = const.tile([B_LOC, CLS], F32)
                        nc.scalar.dma_start(xm2_sb[:], x21_d[:])
                        xm3_sb = const.tile([B_LOC, CLS], F32)
                        nc.scalar.dma_start(xm3_sb[:], x31_d[:])
                        # pre-staged tail tiles (written once, reused)
                        merged = const.tile([B_LOC, 4 * CLS], F32)
                        nc.vector.tensor_copy(merged[:, CLS : 2 * CLS], xm1_sb[:])
                        nc.vector.tensor_copy(
                            merged[:, 2 * CLS : 3 * CLS], xm2_sb[:]
                        )
                        nc.vector.tensor_copy(
                            merged[:, 3 * CLS : 4 * CLS], xm3_sb[:]
                        )
                for item in pend:
                    p1_norm(*item)

            # ===========================================================
            # phase 2: fused fc0+fc1+fc2 -> logits (8, 4) directly; bias
            # folded in as an extra rank-1 chunk with a ones stationary
            # ===========================================================
            with tc.tile_pool(name="p5sb", bufs=1) as sb5, tc.tile_pool(
                name="p5ps", bufs=1, space="PSUM"
            ) as ps5, nc.named_scope("p2_fc012"):
                plog = ps5.tile([B_LOC, CLS], F32, tag="plog")
                for n in range(M):
                    nc.tensor.matmul(
                        plog[:],
                        pallA[:, :, n],
                        w_sb[:, n, :],
                        start=(n == 0),
                        stop=False,
                    )
                for b4 in range(MB):
                    nc.tensor.matmul(
                        plog[:],
                        pallB[:, :, b4],
                        w4_sb[:, b4, :],
                        start=False,
                        stop=False,
                    )
                nc.tensor.matmul(
                    plog[:],
                    ones_col[0:1, 0:B_LOC],
                    bf_sb[:],
                    start=False,
                    stop=True,
                )

                logit_sb = sb5.tile([B_LOC, CLS], F32)
                nc.scalar.copy(logit_sb[:], plog[:])
                nc.sync.dma_start(logits_d[:], logit_sb[:])
                # merged copy on ACT: same queue as the exp below, so no
                # cross-engine semaphore hop on the critical path
                nc.scalar.copy(merged[:, 0:CLS], plog[:])

                # softmax over the 16 features (free dim).  No max-subtract:
                # |merged| <= ~6 here, exp() is safely in range, and softmax
                # is shift-invariant so the result matches the reference.
                # The exp SUM lands in column 16 via accum_out; since the
                # cls map is linear, the softmax division is applied AFTER
                # the matmul:  x_merge = (exp @ cls_w.T + ssum*cls_b)/ssum,
                # with ssum as the 17th feature hitting wct's bias row.
                esb = sb5.tile([B_LOC, 4 * CLS + 1], F32)
                nc.scalar.activation(
                    esb[:, 0 : 4 * CLS],
                    merged[:],
                    mybir.ActivationFunctionType.Exp,
                    accum_out=esb[:, 4 * CLS : 4 * CLS + 1],
                )
                rinv = sb5.tile([B_LOC, 1], F32)
                nc.vector.reciprocal(rinv[:], esb[:, 4 * CLS : 4 * CLS + 1])

                pmt = ps5.tile([4 * CLS + 1, B_LOC], F32, tag="pmt")
                nc.tensor.transpose(pmt[:], esb[:], identf[0:B_LOC, 0:B_LOC])
                mt_aug = sb5.tile([4 * CLS + 1, B_LOC], F32)
                nc.vector.tensor_copy(mt_aug[:], pmt[:])

                pxm = ps5.tile([B_LOC, CLS], F32, tag="pxm")
                nc.tensor.matmul(pxm[:], mt_aug[:], wc_sb[:], start=True, stop=True)
                xm_sb = sb5.tile([B_LOC, CLS], F32)
                nc.vector.tensor_scalar_mul(xm_sb[:], pxm[:], rinv[:])
                nc.sync.dma_start(xmerge_d[:], xm_sb[:])

    return nc


_NC_CACHE = None


def _get_nc():
    global _NC_CACHE
    if _NC_CACHE is None:
        _NC_CACHE = _build_nc()
    return _NC_CACHE


def _make_in_maps(inputs):
    np_mm = np.dtype(mybir.dt.np(MM_DT))

    x1 = np.ascontiguousarray(inputs["x1"], dtype=np.float32).reshape(B, C, L)
    x2 = np.ascontiguousarray(inputs["x2"], dtype=np.float32).reshape(B, C, L)
    x3 = np.ascontiguousarray(inputs["x3"], dtype=np.float32).reshape(B, C, L)
    # (B, L, M) concat + transpose, pad L to 3200, relayout to [B, 128, 25, M]
    xcat = np.concatenate([x1, x2, x3], axis=1).transpose(0, 2, 1)
    xpad = np.zeros((B, LC * 128, M), dtype=np_mm)
    xpad[:, :L] = xcat.astype(np_mm)
    xt_host = np.ascontiguousarray(
        xpad.reshape(B, LC, 128, M).transpose(0, 2, 1, 3)
    )

    x11 = np.ascontiguousarray(inputs["x11"], dtype=np.float32)
    x21 = np.ascontiguousarray(inputs["x21"], dtype=np.float32)
    x31 = np.ascontiguousarray(inputs["x31"], dtype=np.float32)
    fc0_w = np.asarray(inputs["fc0_w"], dtype=np.float32)
    fc0_b = np.asarray(inputs["fc0_b"], dtype=np.float32)
    fc1_w = np.asarray(inputs["fc1_w"], dtype=np.float32)
    fc1_b = np.asarray(inputs["fc1_b"], dtype=np.float32)
    fc2_w = np.asarray(inputs["fc2_w"], dtype=np.float32)
    fc2_b = np.asarray(inputs["fc2_b"], dtype=np.float32)
    cls_w = np.asarray(inputs["cls_w"], dtype=np.float32)
    cls_b = np.asarray(inputs["cls_b"], dtype=np.float32)

    # fuse fc0+fc1+fc2 (all linear; dropout is identity at inference), then
    # apply the symmetric fold (fp32 fold, cast to fp16 at the end)
    Wf = fc2_w @ (fc1_w @ fc0_w)  # (4, 21609)
    bf = (fc2_w @ (fc1_w @ fc0_b + fc1_b) + fc2_b).reshape(1, CLS)
    Wfr = Wf.reshape(CLS, M, M)
    WfA = Wfr[:, :MA, :].copy()
    WfA[:, :, MA:] += Wfr[:, MA:, :MA].transpose(0, 2, 1)
    wA_host = np.ascontiguousarray(WfA.transpose(1, 2, 0).astype(np_mm))  # [m, n, y]
    w4_host = np.ascontiguousarray(
        Wfr[:, MA:, MA:].transpose(1, 2, 0).astype(np_mm)
    )  # [a, b4, y]

    wct = np.ascontiguousarray(
        np.concatenate([cls_w.T, cls_b.reshape(1, CLS)], axis=0)
    )

    in_maps = []
    for c in range(N_CORES):
        sl = slice(B_LOC * c, B_LOC * (c + 1))
        in_maps.append(
            {
                "xall": np.ascontiguousarray(xt_host[sl]),
                "x11": np.ascontiguousarray(x11[sl]),
                "x21": np.ascontiguousarray(x21[sl]),
                "x31": np.ascontiguousarray(x31[sl]),
                "wA": wA_host,
                "w4": w4_host,
                "bf": np.ascontiguousarray(bf),
                "wct": wct,
            }
        )
    return in_maps


def run(inputs, trace=False, **kwargs):
    nc = _get_nc()
    in_maps = _make_in_maps(inputs)
    res = run_bass_kernel_spmd(nc, in_maps, CORE_IDS, trace=trace, **kwargs)
    logits = np.concatenate(
        [np.asarray(res.results[c]["logits"], dtype=np.float32) for c in CORE_IDS]
    )
    x_merge = np.concatenate(
        [np.asarray(res.results[c]["x_merge"], dtype=np.float32) for c in CORE_IDS]
    )
    return (logits, x_merge), res


def kernel(**inputs):
    (logits, x_merge), _ = run(inputs, trace=False)
    return logits, x_merge
